# revision 21
# baseline (speedup 1.0000x reference)
"""CostVolume kernel for Trainium2 (8 NeuronCores, batch-sharded).

out[b,h,w,(di,dj)] = mean_c( prv[b,h,w,c] * nxt_pad[b,h+di,w+dj,c] ),  r=4, d=9.

Device strategy (per core, 2 batches):
  - Host prep: prv scaled by 1/C, block-tiled [b, c, hb, wb, 128] bf16 so each
    (8h x 16w)-pixel block is one contiguous matmul-stationary column set;
    nxt zero-padded by 4 -> [b, c, 136, 136] bf16 (c on SBUF partitions).
  - Per 128-pixel block: two PSUM-accumulated matmuls contract c in 96+96
    chunks (both k=96 round up to the same 128x128 PE tile mode —
    alternating 128/64 modes would drain the array between every matmul).
    Moving rhs = the block's 16x24 shifted window of nxt (j=384 columns);
    stationary = the block's 128 prv pixels; every streamed column serves
    up to 81 outputs.
  - Inputs stream in h-quarters on the sync HWDGE ring (bufs=2 double
    buffering, chunk-A tiles first so compute starts early); DVE + ACT
    alternate draining PSUM->SBUF bf16 two blocks at a time; stores ride
    the ACT HWDGE ring, one DMA per hb-pair ([128, 2*3072] = 12.3 KB
    packets).
  - Host gathers the 81 displacement values per pixel from its block window
    (a per-partition diagonal no lockstep engine can express) and returns
    [B, H, W, 81] f32.
"""

import numpy as np
import ml_dtypes

B, H, W, C = 16, 128, 128, 192
R = 4
D = 2 * R + 1  # 9
N_CORES = 8
B_LOC = B // N_CORES  # 2
CK = 96  # contraction chunk (96+96; round_up_size(96)=128 keeps one PE mode)

HL, WS = 8, 16          # block pixel dims (HL*WS = 128 partitions)
HBN, WBN = H // HL, W // WS  # 16, 8
HP, WP = HL + 2 * R, WS + 2 * R  # window dims 16, 24
J = HP * WP             # 384 moving columns per matmul
HPAD, WPAD = H + 2 * R, W + 2 * R  # 136, 136
NQ = 4                  # h-quarters per batch
HBQ = HBN // NQ         # 4 hblocks per quarter
PRV_QROWS = H // NQ     # 32 prv rows per quarter
NXT_QROWS = PRV_QROWS + 2 * R  # 40 padded nxt rows per quarter

_CACHED = {}


def _build_nc():
    import concourse.mybir as mybir
    from concourse.bacc import Bacc
    from concourse.tile import TileContext

    fp32 = mybir.dt.float32
    bf16 = mybir.dt.bfloat16

    nc = Bacc(
        "TRN2",
        target_bir_lowering=False,
        debug=False,
        num_devices=N_CORES,
    )

    prv_d = nc.dram_tensor(
        "prv_t", [B_LOC, C, HBN, WBN, HL * WS], bf16, kind="ExternalInput"
    )
    nxt_d = nc.dram_tensor(
        "nxt_t", [B_LOC, C, HPAD, WPAD], bf16, kind="ExternalInput"
    )
    x_d = nc.dram_tensor(
        "xband", [B_LOC, HBN // 2, HL * WS, 2 * WBN * J], bf16,
        kind="ExternalOutput"
    )

    with TileContext(nc) as tc:
        with (
            tc.tile_pool(name="prv_pool", bufs=3) as prv_pool,
            tc.tile_pool(name="nxt_pool", bufs=1) as nxt_pool,
            tc.tile_pool(name="x_pool", bufs=4) as x_pool,
            tc.tile_pool(name="psum_pool", bufs=4, space="PSUM") as psum_pool,
        ):
            SEG = HPAD // 4  # 34 padded nxt rows per resident segment
            for b in range(B_LOC):
                # nxt lives in 4 non-overlapping resident segments per chunk
                # (bufs=1: batch b+1's segment load naturally waits for this
                # batch's last reader of that segment — no halo re-reads)
                nsega, nsegb = [], []
                pa0 = pb0 = None
                for s in range(4):
                    ta = nxt_pool.tile([CK, SEG, WPAD], bf16, tag=f"nxt_a{s}")
                    tb = nxt_pool.tile([CK, SEG, WPAD], bf16, tag=f"nxt_b{s}")
                    nc.sync.dma_start(
                        ta[:], nxt_d[b, 0:CK, s * SEG:(s + 1) * SEG, :]
                    )
                    nc.sync.dma_start(
                        tb[:], nxt_d[b, CK:C, s * SEG:(s + 1) * SEG, :]
                    )
                    nsega.append(ta)
                    nsegb.append(tb)
                    if s == 0:
                        # first prv quarter right after segment 0: the ring
                        # is FIFO, so block 0's stationary must not queue
                        # behind the remaining 5.3 MB of segments
                        pa0 = prv_pool.tile(
                            [CK, HBQ, WBN * HL * WS], bf16, tag="prv_a"
                        )
                        pb0 = prv_pool.tile(
                            [CK, HBQ, WBN * HL * WS], bf16, tag="prv_b"
                        )
                        nc.sync.dma_start(pa0[:], prv_d[b, 0:CK, 0:HBQ, :, :])
                        nc.sync.dma_start(pb0[:], prv_d[b, CK:C, 0:HBQ, :, :])

                for q in range(NQ):
                    if q == 0:
                        pa, pb = pa0, pb0
                    else:
                        pa = prv_pool.tile(
                            [CK, HBQ, WBN * HL * WS], bf16, tag="prv_a"
                        )
                        pb = prv_pool.tile(
                            [CK, HBQ, WBN * HL * WS], bf16, tag="prv_b"
                        )
                        hb0 = q * HBQ
                        nc.sync.dma_start(
                            pa[:], prv_d[b, 0:CK, hb0:hb0 + HBQ, :, :]
                        )
                        nc.sync.dma_start(
                            pb[:], prv_d[b, CK:C, hb0:hb0 + HBQ, :, :]
                        )
                    hb0 = q * HBQ

                    for hbp in range(HBQ // 2):
                        xs = x_pool.tile([HL * WS, 2 * WBN * J], bf16, tag="xs")
                        for sub in range(2):
                            hb_loc = hbp * 2 + sub
                            hb = hb0 + hb_loc
                            lo, hi = hb * HL, hb * HL + HP  # padded window rows
                            pieces = []
                            for s in range(4):
                                p0 = max(lo, s * SEG)
                                p1 = min(hi, (s + 1) * SEG)
                                if p0 < p1:
                                    pieces.append((s, p0, p1))
                            npix = HL * WS
                            for wbp in range(WBN // 2):
                                ps = psum_pool.tile(
                                    [HL * WS, 2, 512], fp32, tag="ps"
                                )
                                for k2 in range(2):
                                    wb = wbp * 2 + k2
                                    cc = wb * WS
                                    lhs_sl = slice(wb * npix, (wb + 1) * npix)
                                    n_mm = 2 * len(pieces)
                                    i_mm = 0
                                    for ptile, segs in (
                                        (pa, nsega), (pb, nsegb)
                                    ):
                                        for s, p0, p1 in pieces:
                                            nc.tensor.matmul(
                                                ps[:, k2,
                                                   (p0 - lo) * WP:(p1 - lo) * WP],
                                                ptile[:, hb_loc, lhs_sl],
                                                segs[s][:, p0 - s * SEG:
                                                        p1 - s * SEG,
                                                        cc:cc + WP],
                                                start=(i_mm == 0),
                                                stop=(i_mm == n_mm - 1),
                                            )
                                            i_mm += 1
                                dst = xs[
                                    :,
                                    sub * (WBN * J) + wbp * 2 * J:
                                    sub * (WBN * J) + (wbp * 2 + 2) * J,
                                ]
                                src = ps[:, :, 0:J]
                                if wbp % 2 == 0:
                                    nc.vector.tensor_copy(dst, src)
                                else:
                                    nc.scalar.copy(dst, src)
                            # store this hb's half right after its drains
                            # (ACT HWDGE ring): overlaps the next hb's
                            # compute and halves the end-of-kernel tail
                            nc.scalar.dma_start(
                                x_d[b, (hb0 + hbp * 2) // 2, :,
                                    sub * (WBN * J):(sub + 1) * (WBN * J)],
                                xs[:, sub * (WBN * J):(sub + 1) * (WBN * J)],
                            )

    nc.finalize()
    return nc


def _get_nc():
    if "nc" not in _CACHED:
        _CACHED["nc"] = _build_nc()
    return _CACHED["nc"]


def _host_prep(prv, nxt):
    """prv: scale by 1/C, block-tiled [b, c, hb, wb, 128] bf16.
    nxt: zero-pad by R, [b, c, h+8, w+8] bf16."""
    bf16 = ml_dtypes.bfloat16
    prv_t = (np.asarray(prv, dtype=np.float32) * (1.0 / C)).transpose(0, 3, 1, 2)
    prv_bt = np.ascontiguousarray(
        prv_t.reshape(B, C, HBN, HL, WBN, WS).transpose(0, 1, 2, 4, 3, 5)
    ).reshape(B, C, HBN, WBN, HL * WS).astype(bf16)
    nxt_t = np.zeros((B, C, HPAD, WPAD), dtype=bf16)
    nxt_t[:, :, R:R + H, R:R + W] = np.asarray(nxt, dtype=np.float32).transpose(
        0, 3, 1, 2
    ).astype(bf16)
    return prv_bt, nxt_t


def _gather_x(x):
    """x: [B_LOC, HBN//2, 128, 2*WBN*J] bf16 -> out [B_LOC, H, W, 81] f32."""
    x = np.asarray(x, dtype=np.float32)
    x = x.reshape(B_LOC, HBN // 2, HL * WS, 2, WBN * J).transpose(0, 1, 3, 2, 4)
    x7 = np.ascontiguousarray(x).reshape(B_LOC, HBN, HL, WS, WBN, HP, WP)
    out = np.empty((B_LOC, HBN, HL, WBN, WS, D, D), dtype=np.float32)
    ws_idx = np.arange(WS).reshape(1, 1, WS, 1, 1, 1, 1)
    for hl in range(HL):
        sub = x7[:, :, hl][:, :, :, :, hl:hl + D, :]  # [b, hb, ws, wb, di, WP]
        sw = np.lib.stride_tricks.sliding_window_view(sub, D, axis=5)
        g = np.take_along_axis(sw, ws_idx, axis=5)[:, :, :, :, :, 0, :]
        out[:, :, hl] = g.transpose(0, 1, 3, 2, 4, 5)
    return out.reshape(B_LOC, H, W, D * D)


def _make_in_maps(prv, nxt):
    prv_bt, nxt_t = _host_prep(prv, nxt)
    return [
        {
            "prv_t": prv_bt[i * B_LOC:(i + 1) * B_LOC],
            "nxt_t": nxt_t[i * B_LOC:(i + 1) * B_LOC],
        }
        for i in range(N_CORES)
    ]


def kernel(prv, nxt, search_range):
    from concourse.bass_utils import run_bass_kernel_spmd

    assert int(search_range) == R
    prv = np.asarray(prv)
    nxt = np.asarray(nxt)
    assert prv.shape == (B, H, W, C), prv.shape

    in_maps = _make_in_maps(prv, nxt)
    nc = _get_nc()
    res = run_bass_kernel_spmd(nc, in_maps, list(range(N_CORES)))

    out = np.empty((B, H, W, D * D), dtype=np.float32)
    for i in range(N_CORES):
        out[i * B_LOC:(i + 1) * B_LOC] = _gather_x(res.results[i]["xband"])
    return out
